# revision 64
# baseline (speedup 1.0000x reference)
"""AttentionGCNLayer Trainium2 kernel (fp8 attention + fused softmax den).

Per-sample computation (B=8 samples -> 8 NeuronCores, data-parallel):
  identity = x @ W_it + b_it
  gcn      = relu(adj @ (x @ W_g + b_g))
  h        = LN1(identity + gcn)
  attn     = MHSA(h)  (8 heads, D=32)
  out      = LN2(h + attn)

Design:
  - Host-side: x^T and adj^T are transposed and laid out per-band on the
    host, removing all x/adj PE transposes and their ScalarE PSUM->SBUF
    copies. LN1 gamma folds into W_q/W_k/W_v; k-bias drops (softmax
    invariance); v-bias folds into the output-projection bias. The
    Schraudolph gain A = SCALE*8/ln2 folds into W_q so the DVE exp is a
    single add+min.
  - Softmax exp emitted at fp8e4 with a +24-bit-bias (all exps scaled by
    2^3, cancels in num/den): DVE path bitcasts round(score + 79.4375)
    int8 -> fp8; ScalarE path runs Exp with bias 3*ln2.
  - attn@V runs fp8 DoubleRow over k-chunk PAIRS with per-head weights
    [v | ones]: one matmul per (head, k-pair) yields both the attention
    numerator rows and 32 denominator copies -- the separate denominator
    matmuls of the bf16 design are gone entirely. Normalization uses
    partition-shifted ScalarE copies (den -> numerator partitions) +
    DVE reciprocal + shifted-output muls.
  - Projection + LN2 + store for the first token half drain while the
    second half's attention streams; output ships bf16 and is upcast on
    the host.
"""

import sys

sys.path.insert(0, "/opt/trn_rl_repo")

import numpy as np

import concourse.bass as bass
import concourse.tile as tile
from concourse import bacc, mybir
from concourse.bass_utils import run_bass_kernel_spmd
from concourse.masks import make_identity

F32 = mybir.dt.float32
BF16 = mybir.dt.bfloat16
F8E4 = mybir.dt.float8e4
I8 = mybir.dt.int8
I16 = mybir.dt.int16
I32 = mybir.dt.int32
AF = mybir.ActivationFunctionType
ALU = mybir.AluOpType
PM = mybir.MatmulPerfMode

B, N, CI, CO, H, D = 8, 1024, 128, 256, 8, 32
P = 128
MT = N // P  # 8 token chunks
EPS = 1e-5
SCALE = float(1.0 / np.sqrt(np.float32(D)))
NCORES = 8
MAGIC_P1 = 0x5F3759DF + 1  # quake rsqrt magic + 1 (for the ~t + (M+1) form)

# fp8 Schraudolph: bits = round(score + B8), score pre-scaled by A8 (in W_q).
A8 = float(SCALE * 8.0 / np.log(2.0))
B8 = 79.4375            # (7 - 0.0703)*8 + 24: +24 = global 2^3 on every exp
B8_CLAMP = 126.0        # keep int8 below 127 (0x7F = NaN in e4m3fn)
LN2_8 = float(np.log(2.0) / 8.0)
EXP_ACT_BIAS = float(3.0 * np.log(2.0))  # matches the +24 bit bias (2^3)

# which exp slots go to the DVE: (tp == 1) and k in this set (per group)
DVE_EXP_KS = (0, 1, 2, 3, 4, 5, 6, 7)


def _rsqrt_dve(nc, pool, var_ap, out_ap, consts, n, tag, newton=2):
    """out = 1/sqrt(var + eps) on VectorE only, batched over [128, n].

    Quake bit-trick seed + `newton` Newton iterations (1 iter ~0.17% rel
    err, 2 iters ~5e-6). Keeps ScalarE free of Ln/Sqrt so its activation
    table never switches off Exp.
    """
    eps_sb, sh1_i, neg1_i, magic_i = consts
    xe = pool.tile([P, n], F32, tag=f"rs_xe{tag}")
    nc.vector.tensor_scalar_add(xe, var_ap, eps_sb)
    y = pool.tile([P, n], F32, tag=f"rs_y{tag}")
    ti = pool.tile([P, n], I32, tag=f"rs_ti{tag}")
    # ~(x >> 1)
    nc.vector.tensor_scalar(
        out=ti, in0=xe.bitcast(I32), scalar1=sh1_i, scalar2=neg1_i,
        op0=ALU.logical_shift_right, op1=ALU.bitwise_xor)
    # + (MAGIC+1)  ==  MAGIC - (x >> 1)
    nc.vector.tensor_tensor(
        out=y.bitcast(I32), in0=ti, in1=magic_i.to_broadcast((P, n)), op=ALU.add)
    h = pool.tile([P, n], F32, tag=f"rs_h{tag}")
    nc.vector.tensor_scalar_mul(h, xe, 0.5)
    t2 = pool.tile([P, n], F32, tag=f"rs_t2{tag}")
    for _ in range(newton):
        nc.vector.tensor_mul(t2, y, y)
        nc.vector.tensor_mul(t2, t2, h)
        nc.vector.tensor_scalar(
            out=t2, in0=t2, scalar1=-1.0, scalar2=1.5, op0=ALU.mult, op1=ALU.add)
        nc.vector.tensor_mul(y, y, t2)
    nc.vector.tensor_copy(out_ap, y)


def build_bass(trivial1=True, trivial2=True):
    nc = bacc.Bacc()

    xt_d = nc.dram_tensor("xt", (CI, N), BF16, kind="ExternalInput")
    adjt_d = nc.dram_tensor("adjt", (P, MT * N), BF16, kind="ExternalInput")
    wit_d = nc.dram_tensor("wit", (CI, CO), BF16, kind="ExternalInput")
    wg_d = nc.dram_tensor("wg", (CI, CO), BF16, kind="ExternalInput")
    wq_d = nc.dram_tensor("wq", (CO, CO), BF16, kind="ExternalInput")
    wk_d = nc.dram_tensor("wk", (CO, CO), BF16, kind="ExternalInput")
    wv_d = nc.dram_tensor("wv", (CO, CO), BF16, kind="ExternalInput")
    wo_d = nc.dram_tensor("wo", (CO, CO), BF16, kind="ExternalInput")
    bit_d = nc.dram_tensor("bit", (CO,), F32, kind="ExternalInput")
    bg_d = nc.dram_tensor("bg", (CO,), F32, kind="ExternalInput")
    bq_d = nc.dram_tensor("bq", (CO,), F32, kind="ExternalInput")
    bb2_d = nc.dram_tensor("bb2", (CO,), BF16, kind="ExternalInput")
    if not trivial1:
        g1_d = nc.dram_tensor("g1v", (CO,), F32, kind="ExternalInput")
    if not trivial2:
        g2_d = nc.dram_tensor("g2v", (CO,), F32, kind="ExternalInput")
        be2_d = nc.dram_tensor("be2v", (CO,), F32, kind="ExternalInput")
    out_d = nc.dram_tensor("out", (N, CO), BF16, kind="ExternalOutput")

    with tile.TileContext(nc) as tc:
        from contextlib import ExitStack

        with ExitStack() as ctx:
            singles = ctx.enter_context(tc.tile_pool(name="singles", bufs=1))
            stemp = ctx.enter_context(tc.tile_pool(name="stemp", bufs=3))
            ptemp = ctx.enter_context(tc.tile_pool(name="ptemp", bufs=5))
            adj_pool = ctx.enter_context(tc.tile_pool(name="adj", bufs=4))
            expT_pool = ctx.enter_context(tc.tile_pool(name="expT", bufs=10))
            rec_pool = ctx.enter_context(tc.tile_pool(name="rec", bufs=2))
            ytile_pool = ctx.enter_context(tc.tile_pool(name="ytile", bufs=2))

            # ---------------- Phase -1: big input DMAs first ----------------
            identB = singles.tile([P, P], BF16)
            make_identity(nc, identB)
            # adjt host layout: [p, m, kc, j] = adj[m*128+j, kc*128+p]
            adj_r = adjt_d[:].rearrange("p (m x) -> p m x", m=MT)
            xt_r = xt_d[:].rearrange("c (m t) -> c m t", t=P)
            xT_bf = singles.tile([P, MT, P], BF16)   # x^T  [ci, m, tok]
            # split the x^T load across the three DMA-capable queues so
            # phase 2 gates on ~5us of DMA
            nc.sync.dma_start(xT_bf[:, 0:3, :], xt_r[:, 0:3, :])
            nc.scalar.dma_start(xT_bf[:, 3:6, :], xt_r[:, 3:6, :])
            nc.gpsimd.dma_start(xT_bf[:, 6:8, :], xt_r[:, 6:8, :])
            wg_sb = singles.tile([P, CO], BF16)
            nc.scalar.dma_start(wg_sb, wg_d[:])
            wit_sb = singles.tile([P, CO], BF16)
            nc.gpsimd.dma_start(wit_sb, wit_d[:])

            # ---------------- Phase 0: constants / weights ----------------
            ones1 = singles.tile([1, 512], BF16)
            nc.vector.memset(ones1, 1.0)
            eps_sb = singles.tile([P, 1], F32)
            nc.vector.memset(eps_sb, EPS)
            sh1_i = singles.tile([P, 1], I32)
            nc.vector.memset(sh1_i, 1)
            neg1_i = singles.tile([P, 1], I32)
            nc.vector.memset(neg1_i, -1)
            magic_i = singles.tile([P, 1], I32)
            nc.vector.memset(magic_i, MAGIC_P1)
            consts = (eps_sb, sh1_i, neg1_i, magic_i)
            expb_sb = singles.tile([P, 1], F32)
            nc.vector.memset(expb_sb, EXP_ACT_BIAS)

            def load_w2(dram, name):  # [256,256] -> [128, 2, 256]
                t = singles.tile([P, 2, CO], BF16, tag=f"w2_{name}")
                nc.gpsimd.dma_start(
                    t, dram[:].rearrange("(ko ki) n -> ki ko n", ki=P))
                return t

            def load_row(dram, name):  # [256] -> [1, 256] single-partition row
                t = singles.tile([1, CO], BF16, tag=f"row_{name}")
                nc.gpsimd.dma_start(t, dram[:].rearrange("(a c) -> a c", a=1))
                return t

            def load_bc(dram, name):  # broadcast along partitions: [128, 256]
                t = singles.tile([P, CO], F32, tag=f"bc_{name}")
                src = dram[:]
                bcast = bass.AP(tensor=src.tensor, offset=src.offset,
                                ap=[[0, P]] + list(src.ap))
                nc.gpsimd.dma_start(out=t, in_=bcast)
                return t

            # bias broadcasts + adj^T bands, ordered behind the phase-2
            # gating loads on each queue; attention weights last on gpsimd.
            def load_bc_on(eng, dram, name):
                t = singles.tile([P, CO], F32, tag=f"bc_{name}")
                src = dram[:]
                eng.dma_start(out=t, in_=bass.AP(
                    tensor=src.tensor, offset=src.offset,
                    ap=[[0, P]] + list(src.ap)))
                return t

            bg_bc = load_bc_on(nc.sync, bg_d, "bg")
            bit_bc = load_bc_on(nc.scalar, bit_d, "bit")
            ab_tiles = {}
            for m, eng in zip(range(3), (nc.sync, nc.scalar, nc.gpsimd)):
                ab = adj_pool.tile([P, MT, P], BF16, tag="ab")
                eng.dma_start(
                    ab, adj_r[:, m, :].rearrange("p (k j) -> p k j", j=P))
                ab_tiles[m] = ab
            wq_sb = load_w2(wq_d, "wq")
            wk_sb = load_w2(wk_d, "wk")
            wv_sb = load_w2(wv_d, "wv")
            bq_col = singles.tile([P, 2], F32)   # q bias, per-partition
            nc.gpsimd.dma_start(bq_col, bq_d[:].rearrange("(o i) -> i o", i=P))
            wo_sb = load_w2(wo_d, "wo")
            bb2_row = load_row(bb2_d, "bb2")
            if not trivial1:
                g1_bc = load_bc(g1_d, "g1")
            if not trivial2:
                g2_bc = load_bc(g2_d, "g2")
                be2_bc = load_bc(be2_d, "be2")

            # persistent activations
            t_sb = singles.tile([P, MT, CO], BF16)       # x@W_g + b_g [tok, c]
            s_all = singles.tile([P, MT, CO], F32)       # pre-LN1 residual
            h_bf = singles.tile([P, MT, CO], BF16)       # normalized hhat
            mv_all = singles.tile([P, MT, 2], F32)       # LN1 mean/var
            rstd_all = singles.tile([P, MT], F32)        # LN1 rstd
            nmr_all = singles.tile([P, MT], F32)         # LN1 -mean*rstd
            hT_sb = singles.tile([P, 2, N], BF16)        # hhat^T  [c, tok]
            qT_sb = singles.tile([P, 2, N], BF16)        # q^T     [c, tok]
            kT_sb = singles.tile([P, 2, N], BF16)        # k^T     [c, tok]
            va_sb = singles.tile([P, MT, H, 2 * D], F8E4)  # [v | ones] fp8
            outT_sb = singles.tile([P, 2, N], BF16)      # attn-out^T [c, tok]
            # ones half of va: init whole tile to 1.0, v copies overwrite
            # (gpsimd memset keeps the DVE free)
            nc.gpsimd.memset(va_sb, 1.0)

            # Scores PSUM pool lives across phase 3 AND attention so the
            # first attention slots can interleave with the last gcn
            # chunks (PSUM: htr 1 + mm 2 + sc 4 = 7; qk projections borrow
            # sc tiles).
            sc_ps = ctx.enter_context(
                tc.tile_pool(name="sc_ps", bufs=2, space="PSUM"))
            groups = [(qh, g) for qh in range(2) for g in range(2)]
            group_ex = {}   # gi -> list of [tp0_pair, tp1_pair] per k-pair
            flat = [(gi, k) for gi in range(len(groups)) for k in range(MT)]

            def emit_slot(idx, av=None):
                """Scores + fp8 exp for flat slot idx, with the lagged
                attnV pair's matmuls interleaved BETWEEN the tp score
                pairs: the tp1 scores WAR-wait on the previous slot's exp
                at the PE queue head, so ready attnV work must be queued
                ahead of them or the PE idles."""
                gi, k = flat[idx]
                if k == 0:
                    group_ex[gi] = []
                if k % 2 == 0:
                    # tp0 (ScalarE Exp) writes fp8 dense; tp1 (DVE
                    # Schraudolph) writes i16 (2x mode, 2x faster than the
                    # 1x-mode i8 path) -- attnV reads its low bytes via a
                    # stride-2 fp8 view.
                    ex0 = expT_pool.tile([P, 2, 1024], F8E4, tag="ex")
                    ex1 = expT_pool.tile([P, 2, 1024], I16, tag="exw")
                    group_ex[gi].append([ex0, ex1])
                qh, g = groups[gi]
                qsl = slice(qh * 512, (qh + 1) * 512)
                pair = group_ex[gi][k // 2]
                for tp in range(2):
                    sc = sc_ps.tile([P, 1024], F32, tag="sc")
                    for j2 in range(2):
                        hh = 4 * g + 2 * tp + j2   # global head
                        bp = 32 * (hh % 4)
                        nc.tensor.matmul(
                            sc[:, j2 * 512:(j2 + 1) * 512],
                            kT_sb[bp:bp + 32, g, k * P:(k + 1) * P],
                            qT_sb[bp:bp + 32, g, qsl],
                            start=True, stop=True,
                            tile_position=(bp, 0))
                    dst = pair[tp][:, k % 2, :]
                    if tp == 1 and k in DVE_EXP_KS:
                        nc.vector.tensor_scalar(
                            out=dst, in0=sc,
                            scalar1=B8, scalar2=B8_CLAMP,
                            op0=ALU.add, op1=ALU.min)
                    else:
                        nc.scalar.activation(dst, sc, AF.Exp,
                                             scale=LN2_8,
                                             bias=expb_sb[:, 0:1])
                    if av is not None:
                        emit_av_half(av[0], av[1], tp)

            # slots pre-emittable inside phase 3: (gi0,k0-3) after pair
            # (2,3) lands at m=5; (gi0,k4-5) after pair (4,5) at m=7
            PRE_N = 6

            with ExitStack() as pre:
                htr_ps = pre.enter_context(
                    tc.tile_pool(name="htr_ps", bufs=1, space="PSUM"))
                mm_ps = pre.enter_context(
                    tc.tile_pool(name="mm_ps", bufs=2, space="PSUM"))

                # Warm-up transpose so PE observes the gpsimd sem early.
                warm_ps = htr_ps.tile([P, 2, 2 * P], BF16, tag="htr")
                nc.tensor.transpose(warm_ps[:, 0, 0:P], identB, identB)

                # ---------------- Phase 2: t = x@W_g + b_g (bf16) ------------
                for m in range(MT):
                    tp = mm_ps.tile([P, CO], F32, tag="mm256")
                    nc.tensor.matmul(tp, xT_bf[:, m, :], wg_sb,
                                     start=True, stop=True)
                    nc.vector.tensor_add(t_sb[:, m, :], tp, bg_bc)

                # ---------------- Phase 3: per-chunk gcn + LN1 + qkv ---------
                def emit_hT_qkv(mpair):
                    """h^T transposes + q/k/v projections for a chunk pair.
                    Emitted LAGGED behind the LN1 chain so the PE never
                    head-of-line blocks on DVE statistics."""
                    m0 = mpair[0]
                    psl = slice(m0 * P, (m0 + 2) * P)
                    ps = htr_ps.tile([P, 2, 2 * P], BF16, tag="htr")
                    for kc in range(2):
                        for j, mm in enumerate(mpair):
                            nc.tensor.transpose(
                                ps[:, kc, j * P:(j + 1) * P],
                                h_bf[:, mm, kc * P:(kc + 1) * P], identB)
                    nc.scalar.copy(hT_sb[:, :, psl], ps)
                    # qk projections borrow a scores-pool tile: 4 sections
                    # of 256 = [q oc0 | q oc1 | k oc0 | k oc1]
                    qkp = sc_ps.tile([P, 1024], F32, tag="sc")

                    def sec(a):
                        return qkp[:, a * 2 * P:(a + 1) * 2 * P]

                    for oc in range(2):
                        for kc in range(2):
                            nc.tensor.matmul(
                                sec(oc), wq_sb[:, kc, oc * P:(oc + 1) * P],
                                hT_sb[:, kc, psl],
                                start=(kc == 0), stop=(kc == 1),
                                skip_group_check=True)
                    for oc in range(2):
                        for kc in range(2):
                            nc.tensor.matmul(
                                sec(2 + oc),
                                wk_sb[:, kc, oc * P:(oc + 1) * P],
                                hT_sb[:, kc, psl],
                                start=(kc == 0), stop=(kc == 1),
                                skip_group_check=True)
                    # q bias is per-partition in the qT layout -> fold into
                    # the PSUM->SBUF copy via the ACT bias port
                    for oc in range(2):
                        nc.scalar.activation(
                            qT_sb[:, oc, psl], sec(oc), AF.Identity,
                            bias=bq_col[:, oc:oc + 1])
                    nc.scalar.copy(
                        kT_sb[:, :, psl],
                        qkp[:, 512:1024].rearrange("p (a b) -> p a b", a=2))
                    for mm in mpair:
                        msl = slice(mm * P, (mm + 1) * P)
                        vp = mm_ps.tile([P, CO], F32, tag="mm256")
                        for kc in range(2):
                            nc.tensor.matmul(vp, hT_sb[:, kc, msl],
                                             wv_sb[:, kc, :],
                                             start=(kc == 0), stop=(kc == 1))
                        nc.scalar.copy(
                            va_sb[:, mm, :, 0:D],
                            vp[:].rearrange("p (h d) -> p h d", d=D))

                # adj^T bands arrive straight from DRAM, 3 ahead.
                pending_pair = None
                for m in range(MT):
                    if m + 3 < MT:
                        ab = adj_pool.tile([P, MT, P], BF16, tag="ab")
                        nc.sync.dma_start(
                            ab,
                            adj_r[:, m + 3, :].rearrange("p (k j) -> p k j",
                                                         j=P))
                        ab_tiles[m + 3] = ab
                    at = ab_tiles[m]
                    # identity path for this chunk
                    ip = mm_ps.tile([P, CO], F32, tag="mm256")
                    nc.tensor.matmul(ip, xT_bf[:, m, :], wit_sb,
                                     start=True, stop=True)
                    id_sb = stemp.tile([P, CO], F32, tag="id_sb")
                    nc.vector.tensor_add(id_sb, ip, bit_bc)
                    # gcn chunk
                    gp = mm_ps.tile([P, CO], F32, tag="mm256")
                    for k in range(MT):
                        nc.tensor.matmul(gp, at[:, k, :], t_sb[:, k, :],
                                         start=(k == 0), stop=(k == MT - 1))
                    # s = identity + relu(gcn)
                    nc.vector.scalar_tensor_tensor(
                        out=s_all[:, m, :], in0=gp, scalar=0.0,
                        in1=id_sb, op0=ALU.max, op1=ALU.add)
                    stats = stemp.tile([P, 6], F32, tag="ln_stats")
                    nc.vector.bn_stats(out=stats, in_=s_all[:, m, :])
                    nc.vector.bn_aggr(out=mv_all[:, m, :], in_=stats)
                    if m % 2 == 1:
                        # rsqrt for the (m-1, m) pair; normalize both; emit
                        # the PREVIOUS pair's hT/qkv (two pairs of lag so the
                        # PE never waits on this DVE chain).
                        _rsqrt_dve(nc, stemp, mv_all[:, m - 1:m + 1, 1],
                                   rstd_all[:, m - 1:m + 1], consts, 2, "a",
                                   newton=1)
                        for mm in (m - 1, m):
                            # -mean*rstd (tiny), then the normalize runs on
                            # ScalarE as s*rstd + (-mean*rstd) via the ACT
                            # scale/bias ports (frees the DVE)
                            nc.vector.tensor_scalar(
                                out=nmr_all[:, mm:mm + 1],
                                in0=mv_all[:, mm, 0:1],
                                scalar1=rstd_all[:, mm:mm + 1],
                                scalar2=-1.0, op0=ALU.mult, op1=ALU.mult)
                            nc.scalar.activation(
                                h_bf[:, mm, :], s_all[:, mm, :], AF.Identity,
                                scale=rstd_all[:, mm:mm + 1],
                                bias=nmr_all[:, mm:mm + 1])
                        if pending_pair is not None:
                            emit_hT_qkv(pending_pair)
                        pending_pair = (m - 1, m)
                    # overlap attention startup with the last gcn chunks
                    if m == 6:
                        for idx in range(4):
                            emit_slot(idx)
                    elif m == 7:
                        for idx in range(4, PRE_N):
                            emit_slot(idx)
                emit_hT_qkv(pending_pair)

            # ---------------- Phase 5: attention (group-pipelined) ----------
            # groups: (qh, g) in order; scores+exp of slot i interleave with
            # the fp8 DoubleRow attnV of the k-chunk PAIR finishing at slot
            # i-LAG.
            with ExitStack() as att:
                # PSUM budget (8 banks): sc 2 bufs x [128,1024] = 4, acc 4
                # (each head DoubleRow at [0:64] of its own bank). The
                # projection borrows sc-pool tiles (its chunks slot into
                # the scores buffer rotation).
                acc_ps = att.enter_context(
                    tc.tile_pool(name="acc_ps", bufs=1, space="PSUM"))

                def emit_av_half(gi, p, half):
                    """attn@V+den for k-chunk pair p, heads 2*half and
                    2*half+1: one fp8 DoubleRow matmul per head
                    ([v | ones] x exp pair -> [out | den] rows 0:64 of
                    that head's own bank)."""
                    qh, g = groups[gi]
                    pair = group_ex[gi][p]
                    for j in (2 * half, 2 * half + 1):
                        tp, j2 = j // 2, j % 2
                        hh = 4 * g + j
                        if tp == 0:
                            rhs = pair[0][:, :, j2 * 512:(j2 + 1) * 512]
                        else:
                            # low byte of each i16 = the fp8 exp bits
                            rhs = pair[1][:].bitcast(F8E4).rearrange(
                                "q a (n two) -> q a n two", two=2)[
                                :, :, j2 * 512:(j2 + 1) * 512, 0:1]
                        nc.tensor.matmul(
                            accs[gi][j][0:64, :],
                            va_sb[:, 2 * p:2 * p + 2, hh, :],
                            rhs,
                            start=(p == 0), stop=(p == MT // 2 - 1),
                            perf_mode=PM.DoubleRow)

                def emit_av_pair(gi, p):
                    emit_av_half(gi, p, 0)
                    emit_av_half(gi, p, 1)

                def finish_head(gi, j):
                    """Head j's bank holds [o(0:32) | d(32:64)]. Shifted
                    DVE copy brings the den beside the numerator
                    partitions; GPSIMD quake-reciprocal (1 Newton step,
                    ~0.1% rel err, cancels in softmax scale) + DVE
                    shifted-output mul writes outT row block 32j."""
                    qh, g = groups[gi]
                    qsl = slice(qh * 512, (qh + 1) * 512)
                    acc = accs[gi][j]
                    rec = rec_pool.tile([32, 512], F32, tag=f"rec{j}")
                    nc.scalar.copy(rec, acc[32:64, :])
                    nc.vector.reciprocal_approx_fast(out=rec, in_=rec)
                    nc.vector.tensor_mul(
                        outT_sb[32 * j:32 * j + 32, g, qsl],
                        acc[0:32, :], rec)

                proj_state = {}

                def proj_chunk(qh, i):
                    """Projection + residual + LN2 stats for one chunk.
                    PSUM comes from the sc pool (slots into the scores
                    buffer rotation)."""
                    if i == 0:
                        mv2 = ptemp.tile([P, 4, 2], F32, tag="mv2")
                        proj_state[qh] = ([], mv2)
                    s2s, mv2 = proj_state[qh]
                    m = qh * 4 + i
                    pt = sc_ps.tile([P, 1024], F32, tag="sc")
                    pp = pt[:, 0:CO]
                    nc.tensor.matmul(pp, ones1[:, 0:P], bb2_row,
                                     start=True, stop=False)
                    for cc in range(2):
                        nc.tensor.matmul(
                            pp, outT_sb[:, cc, m * P:(m + 1) * P],
                            wo_sb[:, cc, :],
                            start=False, stop=(cc == 1))
                    # s2 = h*g1 + proj + bb2  (bb2 already in psum)
                    s2 = ptemp.tile([P, CO], F32, tag=f"s2_{i}")
                    if trivial1:
                        nc.vector.tensor_add(s2, pp, h_bf[:, m, :])
                    else:
                        nc.vector.tensor_mul(s2, h_bf[:, m, :], g1_bc)
                        nc.vector.tensor_add(s2, s2, pp)
                    stats = ptemp.tile([P, 6], F32, tag="ln_stats2")
                    nc.vector.bn_stats(out=stats, in_=s2)
                    nc.vector.bn_aggr(out=mv2[:, i, :], in_=stats)
                    s2s.append(s2)

                def proj_norm_store(qh):
                    """LN2 normalize + bf16 store for the 4 chunks of qh;
                    rsqrt runs in two halves so the first stores' DMAs
                    launch earlier, split across queues."""
                    s2s, mv2 = proj_state[qh]
                    engs = (nc.sync, nc.scalar, nc.gpsimd)
                    out_r = out_d[:].rearrange("(mt p) c -> p mt c", p=P)
                    for h2 in range(2):
                        rstd2 = ptemp.tile([P, 2], F32, tag="rstd2")
                        _rsqrt_dve(nc, ptemp, mv2[:, 2 * h2:2 * h2 + 2, 1],
                                   rstd2, consts, 2, "b", newton=1)
                        for i in (2 * h2, 2 * h2 + 1):
                            m = qh * 4 + i
                            i2 = i - 2 * h2
                            yt = ytile_pool.tile([P, CO], BF16)
                            if trivial2:
                                nmr2 = ptemp.tile([P, 1], F32, tag="nmr2")
                                nc.vector.tensor_scalar(
                                    out=nmr2, in0=mv2[:, i, 0:1],
                                    scalar1=rstd2[:, i2:i2 + 1],
                                    scalar2=-1.0, op0=ALU.mult, op1=ALU.mult)
                                nc.scalar.activation(
                                    yt, s2s[i], AF.Identity,
                                    scale=rstd2[:, i2:i2 + 1], bias=nmr2)
                            else:
                                yf = ptemp.tile([P, CO], F32, tag=f"yf_{i}")
                                nc.vector.tensor_scalar(
                                    out=yf, in0=s2s[i],
                                    scalar1=mv2[:, i, 0:1],
                                    scalar2=rstd2[:, i2:i2 + 1],
                                    op0=ALU.subtract, op1=ALU.mult)
                                nc.vector.tensor_mul(yf, yf, g2_bc)
                                nc.vector.tensor_add(yt, yf, be2_bc)
                            # split each chunk store across two queues
                            engs[(2 * i) % 3].dma_start(
                                out_r[0:64, m, :], yt[0:64, :])
                            engs[(2 * i + 1) % 3].dma_start(
                                out_r[64:128, m, :], yt[64:128, :])

                # Flat slot schedule: the attnV pair whose exps land at
                # slot i-LAG runs alongside scores+exp of slot i, so the
                # PE always streams while the exp engines drain. Finish /
                # projection work is queued as SMALL items popped one per
                # slot, so group boundaries never flood the DVE/ScalarE
                # queues ahead of the next slot's exp (which would stall
                # the scores WAR chain). Slots < PRE_N were already
                # emitted inside phase 3; their attnV backlog drains
                # first.
                from collections import deque

                LAG = 3
                accs = {}
                work_q = deque()

                def av_prep(j):
                    """acc alloc + av job for the pair finishing at flat
                    slot j (None if j has no pair)."""
                    if j < 0:
                        return None
                    gj, kj = flat[j]
                    if kj % 2 == 0:
                        return None
                    if kj == 1:
                        # the previous group's finish reads MUST be emitted
                        # before its acc banks are recycled for this group
                        while work_q and work_q[0][0] is finish_head:
                            fn, args = work_q.popleft()
                            fn(*args)
                        a0 = acc_ps.tile([64, 512], F32, tag="acc0")
                        a1 = acc_ps.tile([64, 512], F32, tag="acc1")
                        a2 = acc_ps.tile([64, 512], F32, tag="acc2")
                        a3 = acc_ps.tile([64, 512], F32, tag="acc3")
                        accs[gj] = (a0, a1, a2, a3)
                    return (gj, kj // 2)

                def av_post(j):
                    """group-end bookkeeping after the pair at flat slot j
                    has been emitted."""
                    gj, kj = flat[j]
                    if kj != MT - 1:
                        return
                    for j4 in range(4):
                        work_q.append((finish_head, (gj, j4)))
                    if gj == 1:
                        # outT for qh=0 complete -> drain it while the
                        # qh=1 groups stream.
                        for i4 in range(4):
                            work_q.append((proj_chunk, (0, i4)))
                        work_q.append((proj_norm_store, (0,)))

                for idx in range(len(flat)):
                    av = av_prep(idx - LAG)
                    if idx >= PRE_N:
                        emit_slot(idx, av)
                    elif av is not None:
                        emit_av_pair(*av)
                    if av is not None:
                        av_post(idx - LAG)
                    if work_q:
                        fn, args = work_q.popleft()
                        fn(*args)
                for j in range(len(flat) - LAG, len(flat)):
                    av = av_prep(j)
                    if av is not None:
                        emit_av_pair(*av)
                        av_post(j)
                while work_q:
                    fn, args = work_q.popleft()
                    fn(*args)
                for i4 in range(4):
                    proj_chunk(1, i4)
                proj_norm_store(1)

    nc.finalize()
    return nc


_CACHE = {}


def _get_nc(trivial1, trivial2):
    key = (trivial1, trivial2)
    if key not in _CACHE:
        _CACHE[key] = build_bass(*key)
    return _CACHE[key]


def _prep_host(inputs):
    """Fold LN1 affine + attention biases + the exp gain A8 into weights on
    the host (fp32), cast weights to bf16, and return (shared input map,
    flags)."""
    import ml_dtypes

    BF = ml_dtypes.bfloat16
    f = {k: np.ascontiguousarray(np.asarray(v, np.float32))
         for k, v in inputs.items()}
    g1, be1 = f["g1"], f["beta1"]
    g2, be2 = f["g2"], f["beta2"]
    wq = (g1[:, None] * f["W_q"]) * A8
    bq = (f["b_q"] + be1 @ f["W_q"]) * A8
    wk = g1[:, None] * f["W_k"]
    wv = g1[:, None] * f["W_v"]
    bv = f["b_v"] + be1 @ f["W_v"]
    bb2 = be1 + f["b_o"] + bv @ f["W_o"]

    trivial1 = bool(np.all(g1 == 1.0))
    trivial2 = bool(np.all(g2 == 1.0) and np.all(be2 == 0.0))

    def bf(a):
        return np.ascontiguousarray(a.astype(BF))

    shared = {
        "wit": bf(f["W_it"]), "wg": bf(f["W_g"]),
        "wq": bf(wq), "wk": bf(wk), "wv": bf(wv), "wo": bf(f["W_o"]),
        "bit": f["b_it"], "bg": f["b_g"],
        "bq": bq, "bb2": bf(bb2),
    }
    if not trivial1:
        shared["g1v"] = g1
    if not trivial2:
        shared["g2v"] = g2
        shared["be2v"] = be2
    return shared, trivial1, trivial2


def run(inputs, trace=False):
    shared, trivial1, trivial2 = _prep_host(inputs)
    nc = _get_nc(trivial1, trivial2)
    import ml_dtypes

    BF = ml_dtypes.bfloat16
    x = np.asarray(inputs["x"], np.float32)
    adj = np.asarray(inputs["adj"], np.float32)
    # xt[b]: [ci, m*128+tok] = x[b].T
    xt = np.ascontiguousarray(
        np.transpose(x, (0, 2, 1)).astype(BF))          # [B, CI, N]
    # adjt[b]: [p, m, kc, j] = adj[b][m*128+j, kc*128+p]
    adjt = np.ascontiguousarray(
        adj.reshape(B, MT, P, MT, P)                     # [b, m, j, kc, p]
        .transpose(0, 4, 1, 3, 2)                        # [b, p, m, kc, j]
        .reshape(B, P, MT * N).astype(BF))
    in_maps = []
    for b in range(NCORES):
        m = dict(shared)
        m["xt"] = xt[b]
        m["adjt"] = adjt[b]
        in_maps.append(m)
    res = run_bass_kernel_spmd(nc, in_maps, core_ids=list(range(NCORES)),
                               trace=trace)
    out = np.stack([res.results[b]["out"].astype(np.float32)
                    for b in range(NCORES)], axis=0)
    return out, res


def kernel(**inputs):
    out, _ = run(inputs, trace=False)
    return out


# revision 65
# speedup vs baseline: 1.0389x; 1.0389x over previous
"""AttentionGCNLayer Trainium2 kernel (fp8 attention + fused softmax den).

Per-sample computation (B=8 samples -> 8 NeuronCores, data-parallel):
  identity = x @ W_it + b_it
  gcn      = relu(adj @ (x @ W_g + b_g))
  h        = LN1(identity + gcn)
  attn     = MHSA(h)  (8 heads, D=32)
  out      = LN2(h + attn)

Design:
  - Host-side: x^T and adj^T are transposed and laid out per-band on the
    host, removing all x/adj PE transposes and their ScalarE PSUM->SBUF
    copies. LN1 gamma folds into W_q/W_k/W_v; k-bias drops (softmax
    invariance); v-bias folds into the output-projection bias. The
    Schraudolph gain A = SCALE*8/ln2 folds into W_q so the DVE exp is a
    single add+min.
  - Softmax exp emitted at fp8e4 with a +24-bit-bias (all exps scaled by
    2^3, cancels in num/den): DVE path bitcasts round(score + 79.4375)
    int8 -> fp8; ScalarE path runs Exp with bias 3*ln2.
  - attn@V runs fp8 DoubleRow over k-chunk PAIRS with per-head weights
    [v | ones]: one matmul per (head, k-pair) yields both the attention
    numerator rows and 32 denominator copies -- the separate denominator
    matmuls of the bf16 design are gone entirely. Normalization uses
    partition-shifted ScalarE copies (den -> numerator partitions) +
    DVE reciprocal + shifted-output muls.
  - Projection + LN2 + store for the first token half drain while the
    second half's attention streams; output ships bf16 and is upcast on
    the host.
"""

import sys

sys.path.insert(0, "/opt/trn_rl_repo")

import numpy as np

import concourse.bass as bass
import concourse.tile as tile
from concourse import bacc, mybir
from concourse.bass_utils import run_bass_kernel_spmd
from concourse.masks import make_identity

F32 = mybir.dt.float32
BF16 = mybir.dt.bfloat16
F8E4 = mybir.dt.float8e4
I8 = mybir.dt.int8
I16 = mybir.dt.int16
I32 = mybir.dt.int32
AF = mybir.ActivationFunctionType
ALU = mybir.AluOpType
PM = mybir.MatmulPerfMode

B, N, CI, CO, H, D = 8, 1024, 128, 256, 8, 32
P = 128
MT = N // P  # 8 token chunks
EPS = 1e-5
SCALE = float(1.0 / np.sqrt(np.float32(D)))
NCORES = 8
MAGIC_P1 = 0x5F3759DF + 1  # quake rsqrt magic + 1 (for the ~t + (M+1) form)

# fp8 Schraudolph: bits = round(score + B8), score pre-scaled by A8 (in W_q).
A8 = float(SCALE * 8.0 / np.log(2.0))
B8 = 79.4375            # (7 - 0.0703)*8 + 24: +24 = global 2^3 on every exp
B8_CLAMP = 126.0        # keep int8 below 127 (0x7F = NaN in e4m3fn)
LN2_8 = float(np.log(2.0) / 8.0)
EXP_ACT_BIAS = float(3.0 * np.log(2.0))  # matches the +24 bit bias (2^3)

# which exp slots go to the DVE: (tp == 1) and k in this set (per group)
DVE_EXP_KS = (0, 1, 2, 3, 4, 5, 6, 7)


def _rsqrt_dve(nc, pool, var_ap, out_ap, consts, n, tag, newton=2):
    """out = 1/sqrt(var + eps) on VectorE only, batched over [128, n].

    Quake bit-trick seed + `newton` Newton iterations (1 iter ~0.17% rel
    err, 2 iters ~5e-6). Keeps ScalarE free of Ln/Sqrt so its activation
    table never switches off Exp.
    """
    eps_sb, sh1_i, neg1_i, magic_i = consts
    xe = pool.tile([P, n], F32, tag=f"rs_xe{tag}")
    nc.vector.tensor_scalar_add(xe, var_ap, eps_sb)
    y = pool.tile([P, n], F32, tag=f"rs_y{tag}")
    ti = pool.tile([P, n], I32, tag=f"rs_ti{tag}")
    # ~(x >> 1)
    nc.vector.tensor_scalar(
        out=ti, in0=xe.bitcast(I32), scalar1=sh1_i, scalar2=neg1_i,
        op0=ALU.logical_shift_right, op1=ALU.bitwise_xor)
    # + (MAGIC+1)  ==  MAGIC - (x >> 1)
    nc.vector.tensor_tensor(
        out=y.bitcast(I32), in0=ti, in1=magic_i.to_broadcast((P, n)), op=ALU.add)
    h = pool.tile([P, n], F32, tag=f"rs_h{tag}")
    nc.vector.tensor_scalar_mul(h, xe, 0.5)
    t2 = pool.tile([P, n], F32, tag=f"rs_t2{tag}")
    for _ in range(newton):
        nc.vector.tensor_mul(t2, y, y)
        nc.vector.tensor_mul(t2, t2, h)
        nc.vector.tensor_scalar(
            out=t2, in0=t2, scalar1=-1.0, scalar2=1.5, op0=ALU.mult, op1=ALU.add)
        nc.vector.tensor_mul(y, y, t2)
    nc.vector.tensor_copy(out_ap, y)


def build_bass(trivial1=True, trivial2=True):
    nc = bacc.Bacc()

    xt_d = nc.dram_tensor("xt", (CI, N), BF16, kind="ExternalInput")
    adjt_d = nc.dram_tensor("adjt", (P, MT * N), BF16, kind="ExternalInput")
    wit_d = nc.dram_tensor("wit", (CI, CO), BF16, kind="ExternalInput")
    wg_d = nc.dram_tensor("wg", (CI, CO), BF16, kind="ExternalInput")
    wq_d = nc.dram_tensor("wq", (CO, CO), BF16, kind="ExternalInput")
    wk_d = nc.dram_tensor("wk", (CO, CO), BF16, kind="ExternalInput")
    wv_d = nc.dram_tensor("wv", (CO, CO), BF16, kind="ExternalInput")
    wo_d = nc.dram_tensor("wo", (CO, CO), BF16, kind="ExternalInput")
    bit_d = nc.dram_tensor("bit", (CO,), F32, kind="ExternalInput")
    bg_d = nc.dram_tensor("bg", (CO,), F32, kind="ExternalInput")
    bq_d = nc.dram_tensor("bq", (CO,), F32, kind="ExternalInput")
    bb2_d = nc.dram_tensor("bb2", (CO,), BF16, kind="ExternalInput")
    if not trivial1:
        g1_d = nc.dram_tensor("g1v", (CO,), F32, kind="ExternalInput")
    if not trivial2:
        g2_d = nc.dram_tensor("g2v", (CO,), F32, kind="ExternalInput")
        be2_d = nc.dram_tensor("be2v", (CO,), F32, kind="ExternalInput")
    out_d = nc.dram_tensor("out", (N, CO), BF16, kind="ExternalOutput")

    with tile.TileContext(nc) as tc:
        from contextlib import ExitStack

        with ExitStack() as ctx:
            singles = ctx.enter_context(tc.tile_pool(name="singles", bufs=1))
            stemp = ctx.enter_context(tc.tile_pool(name="stemp", bufs=3))
            ptemp = ctx.enter_context(tc.tile_pool(name="ptemp", bufs=5))
            adj_pool = ctx.enter_context(tc.tile_pool(name="adj", bufs=4))
            expT_pool = ctx.enter_context(tc.tile_pool(name="expT", bufs=10))
            rec_pool = ctx.enter_context(tc.tile_pool(name="rec", bufs=2))
            ytile_pool = ctx.enter_context(tc.tile_pool(name="ytile", bufs=2))

            # ---------------- Phase -1: big input DMAs first ----------------
            identB = singles.tile([P, P], BF16)
            make_identity(nc, identB)
            # adjt host layout: [p, m, kc, j] = adj[m*128+j, kc*128+p]
            adj_r = adjt_d[:].rearrange("p (m x) -> p m x", m=MT)
            xt_r = xt_d[:].rearrange("c (m t) -> c m t", t=P)
            xT_bf = singles.tile([P, MT, P], BF16)   # x^T  [ci, m, tok]
            # split the x^T load across the three DMA-capable queues so
            # phase 2 gates on ~5us of DMA
            nc.sync.dma_start(xT_bf[:, 0:3, :], xt_r[:, 0:3, :])
            nc.scalar.dma_start(xT_bf[:, 3:6, :], xt_r[:, 3:6, :])
            nc.gpsimd.dma_start(xT_bf[:, 6:8, :], xt_r[:, 6:8, :])
            wg_sb = singles.tile([P, CO], BF16)
            nc.scalar.dma_start(wg_sb, wg_d[:])
            wit_sb = singles.tile([P, CO], BF16)
            nc.gpsimd.dma_start(wit_sb, wit_d[:])

            # ---------------- Phase 0: constants / weights ----------------
            ones1 = singles.tile([1, 512], BF16)
            nc.vector.memset(ones1, 1.0)
            eps_sb = singles.tile([P, 1], F32)
            nc.vector.memset(eps_sb, EPS)
            sh1_i = singles.tile([P, 1], I32)
            nc.vector.memset(sh1_i, 1)
            neg1_i = singles.tile([P, 1], I32)
            nc.vector.memset(neg1_i, -1)
            magic_i = singles.tile([P, 1], I32)
            nc.vector.memset(magic_i, MAGIC_P1)
            consts = (eps_sb, sh1_i, neg1_i, magic_i)
            expb_sb = singles.tile([P, 1], F32)
            nc.vector.memset(expb_sb, EXP_ACT_BIAS)

            def load_w2(dram, name):  # [256,256] -> [128, 2, 256]
                t = singles.tile([P, 2, CO], BF16, tag=f"w2_{name}")
                nc.gpsimd.dma_start(
                    t, dram[:].rearrange("(ko ki) n -> ki ko n", ki=P))
                return t

            def load_row(dram, name):  # [256] -> [1, 256] single-partition row
                t = singles.tile([1, CO], BF16, tag=f"row_{name}")
                nc.gpsimd.dma_start(t, dram[:].rearrange("(a c) -> a c", a=1))
                return t

            def load_bc(dram, name):  # broadcast along partitions: [128, 256]
                t = singles.tile([P, CO], F32, tag=f"bc_{name}")
                src = dram[:]
                bcast = bass.AP(tensor=src.tensor, offset=src.offset,
                                ap=[[0, P]] + list(src.ap))
                nc.gpsimd.dma_start(out=t, in_=bcast)
                return t

            # bias broadcasts + adj^T bands, ordered behind the phase-2
            # gating loads on each queue; attention weights last on gpsimd.
            def load_bc_on(eng, dram, name):
                t = singles.tile([P, CO], F32, tag=f"bc_{name}")
                src = dram[:]
                eng.dma_start(out=t, in_=bass.AP(
                    tensor=src.tensor, offset=src.offset,
                    ap=[[0, P]] + list(src.ap)))
                return t

            bg_bc = load_bc_on(nc.sync, bg_d, "bg")
            bit_bc = load_bc_on(nc.scalar, bit_d, "bit")
            ab_tiles = {}
            for m, eng in zip(range(3), (nc.sync, nc.scalar, nc.gpsimd)):
                ab = adj_pool.tile([P, MT, P], BF16, tag="ab")
                eng.dma_start(
                    ab, adj_r[:, m, :].rearrange("p (k j) -> p k j", j=P))
                ab_tiles[m] = ab
            wq_sb = load_w2(wq_d, "wq")
            wk_sb = load_w2(wk_d, "wk")
            wv_sb = load_w2(wv_d, "wv")
            bq_col = singles.tile([P, 2], F32)   # q bias, per-partition
            nc.gpsimd.dma_start(bq_col, bq_d[:].rearrange("(o i) -> i o", i=P))
            wo_sb = load_w2(wo_d, "wo")
            bb2_row = load_row(bb2_d, "bb2")
            if not trivial1:
                g1_bc = load_bc(g1_d, "g1")
            if not trivial2:
                g2_bc = load_bc(g2_d, "g2")
                be2_bc = load_bc(be2_d, "be2")

            # persistent activations
            t_sb = singles.tile([P, MT, CO], BF16)       # x@W_g + b_g [tok, c]
            s_all = singles.tile([P, MT, CO], F32)       # pre-LN1 residual
            h_bf = singles.tile([P, MT, CO], BF16)       # normalized hhat
            mv_all = singles.tile([P, MT, 2], F32)       # LN1 mean/var
            rstd_all = singles.tile([P, MT], F32)        # LN1 rstd
            nmr_all = singles.tile([P, MT], F32)         # LN1 -mean*rstd
            hT_sb = singles.tile([P, 2, N], BF16)        # hhat^T  [c, tok]
            qT_sb = singles.tile([P, 2, N], BF16)        # q^T     [c, tok]
            kT_sb = singles.tile([P, 2, N], BF16)        # k^T     [c, tok]
            va_sb = singles.tile([P, MT, H, 2 * D], F8E4)  # [v | ones] fp8
            outT_sb = singles.tile([P, 2, N], BF16)      # attn-out^T [c, tok]
            # ones half of va: init whole tile to 1.0, v copies overwrite
            # (gpsimd memset keeps the DVE free)
            nc.gpsimd.memset(va_sb, 1.0)

            # Scores PSUM pool lives across phase 3 AND attention so the
            # first attention slots can interleave with the last gcn
            # chunks (PSUM: htr 1 + mm 2 + sc 4 = 7; qk projections borrow
            # sc tiles).
            sc_ps = ctx.enter_context(
                tc.tile_pool(name="sc_ps", bufs=2, space="PSUM"))
            groups = [(qh, g) for qh in range(2) for g in range(2)]
            group_ex = {}   # gi -> list of [tp0_pair, tp1_pair] per k-pair
            flat = [(gi, k) for gi in range(len(groups)) for k in range(MT)]

            def emit_slot(idx, av=None):
                """Scores + fp8 exp for flat slot idx, with the lagged
                attnV pair's matmuls interleaved BETWEEN the tp score
                pairs: the tp1 scores WAR-wait on the previous slot's exp
                at the PE queue head, so ready attnV work must be queued
                ahead of them or the PE idles."""
                gi, k = flat[idx]
                if k == 0:
                    group_ex[gi] = []
                if k % 2 == 0:
                    # tp0 (ScalarE Exp) writes fp8 dense; tp1 (DVE
                    # Schraudolph) writes i16 (2x mode, 2x faster than the
                    # 1x-mode i8 path) -- attnV reads its low bytes via a
                    # stride-2 fp8 view.
                    ex0 = expT_pool.tile([P, 2, 1024], F8E4, tag="ex")
                    ex1 = expT_pool.tile([P, 2, 1024], I16, tag="exw")
                    group_ex[gi].append([ex0, ex1])
                qh, g = groups[gi]
                qsl = slice(qh * 512, (qh + 1) * 512)
                pair = group_ex[gi][k // 2]
                for tp in range(2):
                    sc = sc_ps.tile([P, 1024], F32, tag="sc")
                    for j2 in range(2):
                        hh = 4 * g + 2 * tp + j2   # global head
                        bp = 32 * (hh % 4)
                        nc.tensor.matmul(
                            sc[:, j2 * 512:(j2 + 1) * 512],
                            kT_sb[bp:bp + 32, g, k * P:(k + 1) * P],
                            qT_sb[bp:bp + 32, g, qsl],
                            start=True, stop=True,
                            tile_position=(bp, 0))
                    dst = pair[tp][:, k % 2, :]
                    if tp == 1 and k in DVE_EXP_KS:
                        nc.vector.tensor_scalar(
                            out=dst, in0=sc,
                            scalar1=B8, scalar2=B8_CLAMP,
                            op0=ALU.add, op1=ALU.min)
                    else:
                        nc.scalar.activation(dst, sc, AF.Exp,
                                             scale=LN2_8,
                                             bias=expb_sb[:, 0:1])
                    if av is not None:
                        emit_av_half(av[0], av[1], tp)

            # slots pre-emittable inside phase 3: (gi0,k0-3) after pair
            # (2,3) lands at m=5; (gi0,k4-5) after pair (4,5) at m=7
            PRE_N = 6

            with ExitStack() as pre:
                htr_ps = pre.enter_context(
                    tc.tile_pool(name="htr_ps", bufs=1, space="PSUM"))
                mm_ps = pre.enter_context(
                    tc.tile_pool(name="mm_ps", bufs=2, space="PSUM"))

                # Warm-up transpose so PE observes the gpsimd sem early.
                warm_ps = htr_ps.tile([P, 2, 2 * P], BF16, tag="htr")
                nc.tensor.transpose(warm_ps[:, 0, 0:P], identB, identB)

                # ---------------- Phase 2: t = x@W_g + b_g (bf16) ------------
                for m in range(MT):
                    tp = mm_ps.tile([P, CO], F32, tag="mm256")
                    nc.tensor.matmul(tp, xT_bf[:, m, :], wg_sb,
                                     start=True, stop=True)
                    nc.vector.tensor_add(t_sb[:, m, :], tp, bg_bc)

                # ---------------- Phase 3: per-chunk gcn + LN1 + qkv ---------
                def emit_hT_qkv(mpair):
                    """h^T transposes + q/k/v projections for a chunk pair.
                    Emitted LAGGED behind the LN1 chain so the PE never
                    head-of-line blocks on DVE statistics."""
                    m0 = mpair[0]
                    psl = slice(m0 * P, (m0 + 2) * P)
                    ps = htr_ps.tile([P, 2, 2 * P], BF16, tag="htr")
                    for kc in range(2):
                        for j, mm in enumerate(mpair):
                            nc.tensor.transpose(
                                ps[:, kc, j * P:(j + 1) * P],
                                h_bf[:, mm, kc * P:(kc + 1) * P], identB)
                    nc.scalar.copy(hT_sb[:, :, psl], ps)
                    # qk projections borrow a scores-pool tile: 4 sections
                    # of 256 = [q oc0 | q oc1 | k oc0 | k oc1]
                    qkp = sc_ps.tile([P, 1024], F32, tag="sc")

                    def sec(a):
                        return qkp[:, a * 2 * P:(a + 1) * 2 * P]

                    for oc in range(2):
                        for kc in range(2):
                            nc.tensor.matmul(
                                sec(oc), wq_sb[:, kc, oc * P:(oc + 1) * P],
                                hT_sb[:, kc, psl],
                                start=(kc == 0), stop=(kc == 1),
                                skip_group_check=True)
                    for oc in range(2):
                        for kc in range(2):
                            nc.tensor.matmul(
                                sec(2 + oc),
                                wk_sb[:, kc, oc * P:(oc + 1) * P],
                                hT_sb[:, kc, psl],
                                start=(kc == 0), stop=(kc == 1),
                                skip_group_check=True)
                    # q bias is per-partition in the qT layout -> fold into
                    # the PSUM->SBUF copy via the ACT bias port
                    for oc in range(2):
                        nc.scalar.activation(
                            qT_sb[:, oc, psl], sec(oc), AF.Identity,
                            bias=bq_col[:, oc:oc + 1])
                    nc.scalar.copy(
                        kT_sb[:, :, psl],
                        qkp[:, 512:1024].rearrange("p (a b) -> p a b", a=2))
                    for mm in mpair:
                        msl = slice(mm * P, (mm + 1) * P)
                        vp = mm_ps.tile([P, CO], F32, tag="mm256")
                        for kc in range(2):
                            nc.tensor.matmul(vp, hT_sb[:, kc, msl],
                                             wv_sb[:, kc, :],
                                             start=(kc == 0), stop=(kc == 1))
                        nc.scalar.copy(
                            va_sb[:, mm, :, 0:D],
                            vp[:].rearrange("p (h d) -> p h d", d=D))

                # adj^T bands arrive straight from DRAM, 3 ahead.
                pending_pair = None
                for m in range(MT):
                    if m + 3 < MT:
                        ab = adj_pool.tile([P, MT, P], BF16, tag="ab")
                        nc.sync.dma_start(
                            ab,
                            adj_r[:, m + 3, :].rearrange("p (k j) -> p k j",
                                                         j=P))
                        ab_tiles[m + 3] = ab
                    at = ab_tiles[m]
                    # identity path for this chunk
                    ip = mm_ps.tile([P, CO], F32, tag="mm256")
                    nc.tensor.matmul(ip, xT_bf[:, m, :], wit_sb,
                                     start=True, stop=True)
                    id_sb = stemp.tile([P, CO], F32, tag="id_sb")
                    nc.vector.tensor_add(id_sb, ip, bit_bc)
                    # gcn chunk
                    gp = mm_ps.tile([P, CO], F32, tag="mm256")
                    for k in range(MT):
                        nc.tensor.matmul(gp, at[:, k, :], t_sb[:, k, :],
                                         start=(k == 0), stop=(k == MT - 1))
                    # s = identity + relu(gcn)
                    nc.vector.scalar_tensor_tensor(
                        out=s_all[:, m, :], in0=gp, scalar=0.0,
                        in1=id_sb, op0=ALU.max, op1=ALU.add)
                    stats = stemp.tile([P, 6], F32, tag="ln_stats")
                    nc.vector.bn_stats(out=stats, in_=s_all[:, m, :])
                    nc.vector.bn_aggr(out=mv_all[:, m, :], in_=stats)
                    if m % 2 == 1:
                        # rsqrt for the (m-1, m) pair; normalize both; emit
                        # the PREVIOUS pair's hT/qkv (two pairs of lag so the
                        # PE never waits on this DVE chain).
                        _rsqrt_dve(nc, stemp, mv_all[:, m - 1:m + 1, 1],
                                   rstd_all[:, m - 1:m + 1], consts, 2, "a",
                                   newton=1)
                        for mm in (m - 1, m):
                            nc.vector.tensor_scalar(
                                out=h_bf[:, mm, :], in0=s_all[:, mm, :],
                                scalar1=mv_all[:, mm, 0:1],
                                scalar2=rstd_all[:, mm:mm + 1],
                                op0=ALU.subtract, op1=ALU.mult)
                        if pending_pair is not None:
                            emit_hT_qkv(pending_pair)
                        pending_pair = (m - 1, m)
                    # overlap attention startup with the last gcn chunks
                    if m == 6:
                        for idx in range(4):
                            emit_slot(idx)
                    elif m == 7:
                        for idx in range(4, PRE_N):
                            emit_slot(idx)
                emit_hT_qkv(pending_pair)

            # ---------------- Phase 5: attention (group-pipelined) ----------
            # groups: (qh, g) in order; scores+exp of slot i interleave with
            # the fp8 DoubleRow attnV of the k-chunk PAIR finishing at slot
            # i-LAG.
            with ExitStack() as att:
                # PSUM budget (8 banks): sc 2 bufs x [128,1024] = 4, acc 4
                # (each head DoubleRow at [0:64] of its own bank). The
                # projection borrows sc-pool tiles (its chunks slot into
                # the scores buffer rotation).
                acc_ps = att.enter_context(
                    tc.tile_pool(name="acc_ps", bufs=1, space="PSUM"))

                def emit_av_half(gi, p, half):
                    """attn@V+den for k-chunk pair p, heads 2*half and
                    2*half+1: one fp8 DoubleRow matmul per head
                    ([v | ones] x exp pair -> [out | den] rows 0:64 of
                    that head's own bank)."""
                    qh, g = groups[gi]
                    pair = group_ex[gi][p]
                    for j in (2 * half, 2 * half + 1):
                        tp, j2 = j // 2, j % 2
                        hh = 4 * g + j
                        if tp == 0:
                            rhs = pair[0][:, :, j2 * 512:(j2 + 1) * 512]
                        else:
                            # low byte of each i16 = the fp8 exp bits
                            rhs = pair[1][:].bitcast(F8E4).rearrange(
                                "q a (n two) -> q a n two", two=2)[
                                :, :, j2 * 512:(j2 + 1) * 512, 0:1]
                        nc.tensor.matmul(
                            accs[gi][j][0:64, :],
                            va_sb[:, 2 * p:2 * p + 2, hh, :],
                            rhs,
                            start=(p == 0), stop=(p == MT // 2 - 1),
                            perf_mode=PM.DoubleRow)

                def emit_av_pair(gi, p):
                    emit_av_half(gi, p, 0)
                    emit_av_half(gi, p, 1)

                def finish_head(gi, j):
                    """Head j's bank holds [o(0:32) | d(32:64)]. Shifted
                    DVE copy brings the den beside the numerator
                    partitions; GPSIMD quake-reciprocal (1 Newton step,
                    ~0.1% rel err, cancels in softmax scale) + DVE
                    shifted-output mul writes outT row block 32j."""
                    qh, g = groups[gi]
                    qsl = slice(qh * 512, (qh + 1) * 512)
                    acc = accs[gi][j]
                    rec = rec_pool.tile([32, 512], F32, tag=f"rec{j}")
                    nc.scalar.copy(rec, acc[32:64, :])
                    nc.vector.reciprocal_approx_fast(out=rec, in_=rec)
                    nc.vector.tensor_mul(
                        outT_sb[32 * j:32 * j + 32, g, qsl],
                        acc[0:32, :], rec)

                proj_state = {}

                def proj_chunk(qh, i):
                    """Projection + residual + LN2 stats for one chunk.
                    PSUM comes from the sc pool (slots into the scores
                    buffer rotation)."""
                    if i == 0:
                        mv2 = ptemp.tile([P, 4, 2], F32, tag="mv2")
                        proj_state[qh] = ([], mv2)
                    s2s, mv2 = proj_state[qh]
                    m = qh * 4 + i
                    pt = sc_ps.tile([P, 1024], F32, tag="sc")
                    pp = pt[:, 0:CO]
                    nc.tensor.matmul(pp, ones1[:, 0:P], bb2_row,
                                     start=True, stop=False)
                    for cc in range(2):
                        nc.tensor.matmul(
                            pp, outT_sb[:, cc, m * P:(m + 1) * P],
                            wo_sb[:, cc, :],
                            start=False, stop=(cc == 1))
                    # s2 = h*g1 + proj + bb2  (bb2 already in psum)
                    s2 = ptemp.tile([P, CO], F32, tag=f"s2_{i}")
                    if trivial1:
                        nc.vector.tensor_add(s2, pp, h_bf[:, m, :])
                    else:
                        nc.vector.tensor_mul(s2, h_bf[:, m, :], g1_bc)
                        nc.vector.tensor_add(s2, s2, pp)
                    stats = ptemp.tile([P, 6], F32, tag="ln_stats2")
                    nc.vector.bn_stats(out=stats, in_=s2)
                    nc.vector.bn_aggr(out=mv2[:, i, :], in_=stats)
                    s2s.append(s2)

                def proj_norm_store(qh):
                    """LN2 normalize + bf16 store for the 4 chunks of qh;
                    rsqrt runs in two halves so the first stores' DMAs
                    launch earlier, split across queues."""
                    s2s, mv2 = proj_state[qh]
                    engs = (nc.sync, nc.scalar, nc.gpsimd)
                    out_r = out_d[:].rearrange("(mt p) c -> p mt c", p=P)
                    for h2 in range(2):
                        rstd2 = ptemp.tile([P, 2], F32, tag="rstd2")
                        _rsqrt_dve(nc, ptemp, mv2[:, 2 * h2:2 * h2 + 2, 1],
                                   rstd2, consts, 2, "b", newton=1)
                        for i in (2 * h2, 2 * h2 + 1):
                            m = qh * 4 + i
                            i2 = i - 2 * h2
                            yt = ytile_pool.tile([P, CO], BF16)
                            if trivial2:
                                nmr2 = ptemp.tile([P, 1], F32, tag="nmr2")
                                nc.vector.tensor_scalar(
                                    out=nmr2, in0=mv2[:, i, 0:1],
                                    scalar1=rstd2[:, i2:i2 + 1],
                                    scalar2=-1.0, op0=ALU.mult, op1=ALU.mult)
                                nc.scalar.activation(
                                    yt, s2s[i], AF.Identity,
                                    scale=rstd2[:, i2:i2 + 1], bias=nmr2)
                            else:
                                yf = ptemp.tile([P, CO], F32, tag=f"yf_{i}")
                                nc.vector.tensor_scalar(
                                    out=yf, in0=s2s[i],
                                    scalar1=mv2[:, i, 0:1],
                                    scalar2=rstd2[:, i2:i2 + 1],
                                    op0=ALU.subtract, op1=ALU.mult)
                                nc.vector.tensor_mul(yf, yf, g2_bc)
                                nc.vector.tensor_add(yt, yf, be2_bc)
                            # split each chunk store across two queues
                            engs[(2 * i) % 3].dma_start(
                                out_r[0:64, m, :], yt[0:64, :])
                            engs[(2 * i + 1) % 3].dma_start(
                                out_r[64:128, m, :], yt[64:128, :])

                # Flat slot schedule: the attnV pair whose exps land at
                # slot i-LAG runs alongside scores+exp of slot i, so the
                # PE always streams while the exp engines drain. Finish /
                # projection work is queued as SMALL items popped one per
                # slot, so group boundaries never flood the DVE/ScalarE
                # queues ahead of the next slot's exp (which would stall
                # the scores WAR chain). Slots < PRE_N were already
                # emitted inside phase 3; their attnV backlog drains
                # first.
                from collections import deque

                LAG = 3
                accs = {}
                work_q = deque()

                def av_prep(j):
                    """acc alloc + av job for the pair finishing at flat
                    slot j (None if j has no pair)."""
                    if j < 0:
                        return None
                    gj, kj = flat[j]
                    if kj % 2 == 0:
                        return None
                    if kj == 1:
                        # the previous group's finish reads MUST be emitted
                        # before its acc banks are recycled for this group
                        while work_q and work_q[0][0] is finish_head:
                            fn, args = work_q.popleft()
                            fn(*args)
                        a0 = acc_ps.tile([64, 512], F32, tag="acc0")
                        a1 = acc_ps.tile([64, 512], F32, tag="acc1")
                        a2 = acc_ps.tile([64, 512], F32, tag="acc2")
                        a3 = acc_ps.tile([64, 512], F32, tag="acc3")
                        accs[gj] = (a0, a1, a2, a3)
                    return (gj, kj // 2)

                def av_post(j):
                    """group-end bookkeeping after the pair at flat slot j
                    has been emitted."""
                    gj, kj = flat[j]
                    if kj != MT - 1:
                        return
                    for j4 in range(4):
                        work_q.append((finish_head, (gj, j4)))
                    if gj == 1:
                        # outT for qh=0 complete -> drain it while the
                        # qh=1 groups stream.
                        for i4 in range(4):
                            work_q.append((proj_chunk, (0, i4)))
                        work_q.append((proj_norm_store, (0,)))

                for idx in range(len(flat)):
                    av = av_prep(idx - LAG)
                    if idx >= PRE_N:
                        emit_slot(idx, av)
                    elif av is not None:
                        emit_av_pair(*av)
                    if av is not None:
                        av_post(idx - LAG)
                    if work_q:
                        fn, args = work_q.popleft()
                        fn(*args)
                for j in range(len(flat) - LAG, len(flat)):
                    av = av_prep(j)
                    if av is not None:
                        emit_av_pair(*av)
                        av_post(j)
                while work_q:
                    fn, args = work_q.popleft()
                    fn(*args)
                for i4 in range(4):
                    proj_chunk(1, i4)
                proj_norm_store(1)

    nc.finalize()
    return nc


_CACHE = {}


def _get_nc(trivial1, trivial2):
    key = (trivial1, trivial2)
    if key not in _CACHE:
        _CACHE[key] = build_bass(*key)
    return _CACHE[key]


def _prep_host(inputs):
    """Fold LN1 affine + attention biases + the exp gain A8 into weights on
    the host (fp32), cast weights to bf16, and return (shared input map,
    flags)."""
    import ml_dtypes

    BF = ml_dtypes.bfloat16
    f = {k: np.ascontiguousarray(np.asarray(v, np.float32))
         for k, v in inputs.items()}
    g1, be1 = f["g1"], f["beta1"]
    g2, be2 = f["g2"], f["beta2"]
    wq = (g1[:, None] * f["W_q"]) * A8
    bq = (f["b_q"] + be1 @ f["W_q"]) * A8
    wk = g1[:, None] * f["W_k"]
    wv = g1[:, None] * f["W_v"]
    bv = f["b_v"] + be1 @ f["W_v"]
    bb2 = be1 + f["b_o"] + bv @ f["W_o"]

    trivial1 = bool(np.all(g1 == 1.0))
    trivial2 = bool(np.all(g2 == 1.0) and np.all(be2 == 0.0))

    def bf(a):
        return np.ascontiguousarray(a.astype(BF))

    shared = {
        "wit": bf(f["W_it"]), "wg": bf(f["W_g"]),
        "wq": bf(wq), "wk": bf(wk), "wv": bf(wv), "wo": bf(f["W_o"]),
        "bit": f["b_it"], "bg": f["b_g"],
        "bq": bq, "bb2": bf(bb2),
    }
    if not trivial1:
        shared["g1v"] = g1
    if not trivial2:
        shared["g2v"] = g2
        shared["be2v"] = be2
    return shared, trivial1, trivial2


def run(inputs, trace=False):
    shared, trivial1, trivial2 = _prep_host(inputs)
    nc = _get_nc(trivial1, trivial2)
    import ml_dtypes

    BF = ml_dtypes.bfloat16
    x = np.asarray(inputs["x"], np.float32)
    adj = np.asarray(inputs["adj"], np.float32)
    # xt[b]: [ci, m*128+tok] = x[b].T
    xt = np.ascontiguousarray(
        np.transpose(x, (0, 2, 1)).astype(BF))          # [B, CI, N]
    # adjt[b]: [p, m, kc, j] = adj[b][m*128+j, kc*128+p]
    adjt = np.ascontiguousarray(
        adj.reshape(B, MT, P, MT, P)                     # [b, m, j, kc, p]
        .transpose(0, 4, 1, 3, 2)                        # [b, p, m, kc, j]
        .reshape(B, P, MT * N).astype(BF))
    in_maps = []
    for b in range(NCORES):
        m = dict(shared)
        m["xt"] = xt[b]
        m["adjt"] = adjt[b]
        in_maps.append(m)
    res = run_bass_kernel_spmd(nc, in_maps, core_ids=list(range(NCORES)),
                               trace=trace)
    out = np.stack([res.results[b]["out"].astype(np.float32)
                    for b in range(NCORES)], axis=0)
    return out, res


def kernel(**inputs):
    out, _ = run(inputs, trace=False)
    return out
